# revision 17
# baseline (speedup 1.0000x reference)
"""Trainium2 Bass kernel for nn_ContractiveLoss (triplet + pairwise-cosine MSE loss).

Math:
  triplet = mean(relu(||a-p+eps|| - ||a-n+eps|| + margin))
  sim     = mean((A_hat A_hat^T - S)^2),  A_hat = anchor rows normalized

The B x B cosine matrix is never materialized. Using
  sum((cos - S)^2) = sum(cos^2) - 2*sum(cos*S) + sum(S^2)
with
  sum(cos^2)  = ||G||_F^2,  G = A_hat^T A_hat            (D x D Gram)
  sum(cos*S)  = <Q^T, A_hat^T> where Q^T = A_hat_loc^T S  (PE matmuls)
  sum(S^2)    = elementwise square+accumulate over S tiles
Work is sharded row-wise across 8 NeuronCores; small partials combined on host.

Optimizations vs the first working version (59.9 us):
  * All large inputs are pre-cast on the HOST so HBM traffic shrinks:
      S shard    fp32 -> fp8e4   (33.6 MB -> 8.4 MB per core)
      anchor^T   fp32 -> fp8e4   (8.4 MB full-anchor read -> 2.1 MB)
      locals     fp32 -> bf16    (3.1 MB -> 1.6 MB)
    Plain (no-cast) DMAs also use the faster HWDGE path.
  * The big matmul Q^T = A_hat_loc^T @ S runs in fp8 DoubleRow perf mode
    (2 fp8 weights/cell, contraction 256/pass): A_hat_local is the
    stationary operand (reused across 4+ moving passes) and S streams at
    N=512, halving PE busy time vs bf16.
  * The full-anchor row norms are computed in transposed layout: square
    a^T, reduce over partitions with a ones-stationary matmul (PE),
    sqrt on ACT, reciprocal on DVE, all in [128 x 512] chunks.  No
    Newton step: fp8 quantization noise dominates anyway (host-validated
    rel err 3.4e-4 vs 2e-2 budget).
  * The 8.4M-element sum(S^2) is split across ACT, GpSimd and DVE
    column-slices instead of running serially on ACT.

build(..., repeat=K) emits the body K times into one NEFF - used only for
timing (per-iteration steady-state period).
"""

import numpy as np

import concourse.bacc as bacc
import concourse.mybir as mybir
from concourse.tile import TileContext

F32 = mybir.dt.float32
BF16 = mybir.dt.bfloat16
FP8 = mybir.dt.float8e4
AL = mybir.AluOpType
AF = mybir.ActivationFunctionType
PM = mybir.MatmulPerfMode

MARGIN = 0.2
PD_EPS = 1e-6
COS_EPS = 1e-8

B_FULL, D_FULL, NCORES = 8192, 256, 8

# sum(S^2) column split per 2048-wide group: ACT / DVE
SQ_ACT, SQ_DVE = 1536, 512

_cache = {}


def _newton_sqrt(nc, scr_pool, y, x, cols):
    """y[:, cols] = sqrt(x[:, cols]), ACT sqrt + one Newton step (packed)."""
    p, n = y.shape[0], cols.stop - cols.start
    r = scr_pool.tile([p, n], F32, tag="nsq_r")
    nc.scalar.activation(out=y[:, cols], in_=x[:, cols], func=AF.Sqrt)
    nc.vector.reciprocal(out=r, in_=y[:, cols])
    nc.vector.tensor_mul(out=r, in0=r, in1=x[:, cols])
    nc.vector.scalar_tensor_tensor(
        out=y[:, cols], in0=y[:, cols], scalar=1.0, in1=r,
        op0=AL.mult, op1=AL.add,
    )
    nc.vector.tensor_scalar_mul(out=y[:, cols], in0=y[:, cols], scalar1=0.5)


def build(B, D, ncores, repeat=1):
    """Build the per-core SPMD Bass module (identical NEFF on all cores)."""
    R = B // ncores          # local rows per core
    LT = R // 128            # local 128-row tiles
    T4 = LT // 2             # local row-tile PAIRS (DoubleRow contraction 256)
    JW = min(2048, B)        # S column-group width (one DMA each)
    NG = B // JW             # number of S groups
    JC = JW // 512           # 512-col j-chunks per group
    NH = D // 128            # D chunks (psum partition groups)
    assert R % 256 == 0 and D % 128 == 0 and B % JW == 0 and JW % 512 == 0
    QD = B // 512            # qdot partial columns (replicated over partitions)
    NS2 = NG * 2             # sum(S^2) partial columns
    MC = QD + NS2 + 1        # + triplet
    sq_splits = []
    c0 = 0
    for w in (SQ_ACT * JW // 2048, SQ_DVE * JW // 2048):
        sq_splits.append((c0, c0 + w))
        c0 += w
    assert c0 == JW

    nc = bacc.Bacc("TRN2")
    a_t = nc.dram_tensor("a_t", [D, B], FP8, kind="ExternalInput")
    anchor_l = nc.dram_tensor("anchor_local", [R, D], BF16, kind="ExternalInput")
    pos = nc.dram_tensor("pos", [R, D], BF16, kind="ExternalInput")
    neg = nc.dram_tensor("neg", [R, D], BF16, kind="ExternalInput")
    s = nc.dram_tensor("s", [R, B], FP8, kind="ExternalInput")
    g_out = nc.dram_tensor("g_out", [D, D], F32, kind="ExternalOutput")
    misc_out = nc.dram_tensor("misc_out", [128, MC], F32, kind="ExternalOutput")

    with TileContext(nc) as tc:
        with (
            tc.tile_pool(name="singles", bufs=1) as singles,
            tc.tile_pool(name="stiles", bufs=3) as s_pool,
            tc.tile_pool(name="scr", bufs=4) as scr_pool,
            tc.tile_pool(name="nrm", bufs=3) as nrm_pool,
            tc.tile_pool(name="ptile", bufs=4) as p_pool,
            tc.tile_pool(name="qpsum", bufs=4, space="PSUM") as q_psum,
            tc.tile_pool(name="spsum", bufs=1, space="PSUM") as s_psum,
            tc.tile_pool(name="gpsum", bufs=1, space="PSUM") as g_psum,
        ):
            # persistent tiles (shared across repeats)
            misc = singles.tile([128, MC], F32)
            at = singles.tile([128, NH, B], FP8)       # anchor^T
            at2 = singles.tile([128, NH, B], FP8)      # (anchor^T)^2
            inv_all = singles.tile([128, B // 512, 512], mybir.dt.float16)
            ones8 = singles.tile([128, 2, 128], FP8)
            al = singles.tile([128, LT, D], BF16)
            albf = singles.tile([128, LT, D], FP8)     # A_hat local (weights)
            pt = singles.tile([128, LT, D], BF16)
            nt_ = singles.tile([128, LT, D], BF16)
            ssql = singles.tile([128, LT], F32)
            nrml = singles.tile([128, LT], F32)
            invl = singles.tile([128, LT], F32)
            dp2 = singles.tile([128, LT], F32)
            dn2 = singles.tile([128, LT], F32)
            dpt = singles.tile([128, LT], F32)
            dnt = singles.tile([128, LT], F32)
            tm = singles.tile([128, LT], F32)
            rlu = singles.tile([128, LT], F32)
            g_sb = singles.tile([128, NH, D], F32)
            epsb = singles.tile([128, 1], F32)
            nc.vector.memset(epsb, PD_EPS)
            nc.vector.memset(ones8, 1.0)

            viewA = a_t[:, :].rearrange("(h p) j -> p h j", p=128)
            viewS = s[:, :].rearrange("(t p) j -> p t j", p=128)

            for _rep in range(repeat):
                # ---------------- input DMAs (plain copies, HWDGE) --------
                nc.sync.dma_start(
                    out=al,
                    in_=anchor_l[:, :].rearrange("(t p) d -> p t d", p=128))
                nc.sync.dma_start(out=at, in_=viewA)
                sts = []
                for g in range(NG):
                    st = s_pool.tile([128, LT, JW], FP8, tag=f"st")
                    eng = nc.sync if g % 2 == 0 else nc.scalar
                    eng.dma_start(
                        out=st, in_=viewS[:, :, g * JW:(g + 1) * JW])
                    sts.append(st)
                nc.scalar.dma_start(
                    out=pt, in_=pos[:, :].rearrange("(t p) d -> p t d", p=128))
                nc.scalar.dma_start(
                    out=nt_, in_=neg[:, :].rearrange("(t p) d -> p t d", p=128))

                # ---------------- local prep: norms + A_hat_local ---------
                colsl = slice(0, LT)
                for i in range(LT):
                    sc = scr_pool.tile([128, D], F32, tag="ssq_scr")
                    nc.vector.scalar_tensor_tensor(
                        out=sc, in0=al[:, i, :], scalar=0.0, in1=al[:, i, :],
                        op0=AL.bypass, op1=AL.mult,
                        accum_out=ssql[:, i:i + 1])
                _newton_sqrt(nc, scr_pool, nrml, ssql, colsl)
                nc.vector.tensor_scalar_max(out=nrml, in0=nrml, scalar1=COS_EPS)
                nc.vector.reciprocal(out=invl, in_=nrml)
                for i in range(LT):
                    nc.vector.tensor_scalar_mul(
                        out=albf[:, i, :], in0=al[:, i, :],
                        scalar1=invl[:, i:i + 1])

                # ---------------- triplet (ACT squares + packed sqrt) -----
                for i in range(LT):
                    for (other, acc) in ((pt, dp2), (nt_, dn2)):
                        sc = scr_pool.tile([128, D], F32)
                        nc.vector.tensor_sub(
                            out=sc, in0=al[:, i, :], in1=other[:, i, :])
                        sc2 = scr_pool.tile([128, D], F32)
                        nc.scalar.activation(
                            out=sc2, in_=sc, func=AF.Square, bias=epsb[:, :],
                            scale=1.0, accum_out=acc[:, i:i + 1])
                _newton_sqrt(nc, scr_pool, dpt, dp2, colsl)
                _newton_sqrt(nc, scr_pool, dnt, dn2, colsl)
                nc.vector.scalar_tensor_tensor(
                    out=tm, in0=dpt, scalar=MARGIN, in1=dnt,
                    op0=AL.add, op1=AL.subtract)
                nc.vector.tensor_scalar(
                    out=rlu, in0=tm, scalar1=0.0, scalar2=None, op0=AL.max,
                    op1=AL.add, accum_out=misc[:, QD + NS2:QD + NS2 + 1])

                # ---------------- local Gram G_c = Albf^T Albf (fp8 DR) ---
                for h in range(NH):
                    gps = g_psum.tile([128, D], F32)
                    for t in range(T4):
                        nc.tensor.matmul(
                            out=gps,
                            lhsT=albf[:, 2 * t:2 * t + 2, h * 128:(h + 1) * 128],
                            rhs=albf[:, 2 * t:2 * t + 2, :],
                            start=(t == 0), stop=(t == T4 - 1),
                            perf_mode=PM.DoubleRow)
                    nc.vector.tensor_copy(out=g_sb[:, h, :], in_=gps)
                nc.sync.dma_start(
                    out=g_out[:, :].rearrange("(h p) k -> p h k", p=128),
                    in_=g_sb)

                # ---------------- main loop over S column groups ----------
                for g in range(NG):
                    st = sts[g]
                    # (a) square a^T columns of this group (ACT)
                    for h in range(NH):
                        cols = slice(g * JW, (g + 1) * JW)
                        nc.scalar.square(
                            out=at2[:, h, cols], in_=at[:, h, cols])
                    for jc in range(JC):
                        j0 = g * JW + jc * 512
                        cols = slice(j0, j0 + 512)
                        ci = j0 // 512
                        # (b) column ssq via ones-matmul (partition reduce,
                        #     DoubleRow sums both 128-row halves in one pass)
                        sps = s_psum.tile([128, 512], F32)
                        nc.tensor.matmul(
                            out=sps, lhsT=ones8, rhs=at2[:, :, cols],
                            start=True, stop=True, perf_mode=PM.DoubleRow)
                        # (c) norm -> inv (replicated layout, no Newton)
                        nrm = nrm_pool.tile([128, 512], F32, tag="nrm")
                        nc.scalar.activation(out=nrm, in_=sps, func=AF.Sqrt)
                        with nc.allow_low_precision(
                                reason="fp16 1/norm; fp8 noise dominates"):
                            nc.vector.reciprocal(
                                out=inv_all[:, ci, :], in_=nrm)

                    # (e) Q^T = A_hat_loc^T @ S  (fp8 DoubleRow, N=512)
                    for jc in range(JC):
                        j0 = g * JW + jc * 512
                        cols = slice(j0, j0 + 512)
                        ci = j0 // 512
                        ptile = p_pool.tile([128, 2, 512], FP8, tag="p")
                        for h in range(NH):
                            qps = q_psum.tile([128, 512], F32)
                            for t in range(T4):
                                nc.tensor.matmul(
                                    out=qps,
                                    lhsT=albf[:, 2 * t:2 * t + 2,
                                              h * 128:(h + 1) * 128],
                                    rhs=st[:, 2 * t:2 * t + 2,
                                           jc * 512:(jc + 1) * 512],
                                    start=(t == 0), stop=(t == T4 - 1),
                                    perf_mode=PM.DoubleRow)
                            # (f) P = Q^T * a^T (raw, un-normalized)
                            nc.vector.scalar_tensor_tensor(
                                out=ptile[:, h, :], in0=qps, scalar=0.0,
                                in1=at[:, h, cols],
                                op0=AL.bypass, op1=AL.mult)
                        # (g) u_j = sum_k P[k, j] (ones-matmul), then
                        #     qdot partial = sum_j u_j * inv_j
                        ups = s_psum.tile([128, 512], F32)
                        nc.tensor.matmul(
                            out=ups, lhsT=ones8, rhs=ptile[:, :, :],
                            start=True, stop=True, perf_mode=PM.DoubleRow)
                        qsc = scr_pool.tile([128, 512], F32, tag="qdot")
                        nc.vector.scalar_tensor_tensor(
                            out=qsc, in0=ups, scalar=0.0,
                            in1=inv_all[:, ci, :],
                            op0=AL.bypass, op1=AL.mult,
                            accum_out=misc[:, ci:ci + 1])

                    # (g) sum(S^2) for this group, split across engines
                    for e, (e0, e1) in enumerate(sq_splits):
                        col = QD + g * 2 + e
                        acc = misc[:, col:col + 1]
                        sl = st[:, :, e0:e1]
                        if e == 0:
                            nc.scalar.activation(
                                out=sl, in_=sl, func=AF.Square, accum_out=acc)
                        else:
                            nc.vector.scalar_tensor_tensor(
                                out=sl, in0=sl, scalar=0.0, in1=sl,
                                op0=AL.bypass, op1=AL.mult, accum_out=acc)

                nc.sync.dma_start(out=misc_out[:, :], in_=misc)

    nc.finalize()
    return nc


def _get_nc(B, D, ncores, repeat=1):
    key = (B, D, ncores, repeat)
    if key not in _cache:
        _cache[key] = build(B, D, ncores, repeat=repeat)
    return _cache[key]


_jit_cache = {}


def _make_jit(nc, n_cores):
    """Build a cached sharded jit around the bass_exec custom call."""
    import jax
    from jax.sharding import Mesh, PartitionSpec
    try:
        from jax.experimental.shard_map import shard_map
    except ImportError:
        from jax import shard_map
    import concourse.bass2jax as bass2jax

    bass2jax.install_neuronx_cc_hook()
    partition_name = (nc.partition_id_tensor.name
                      if nc.partition_id_tensor else None)
    in_names, out_names, out_avals = [], [], []
    for alloc in nc.m.functions[0].allocations:
        if not isinstance(alloc, mybir.MemoryLocationSet):
            continue
        name = alloc.memorylocations[0].name
        if alloc.kind == "ExternalInput":
            if name != partition_name:
                in_names.append(name)
        elif alloc.kind == "ExternalOutput":
            out_names.append(name)
            out_avals.append(jax.core.ShapedArray(
                tuple(alloc.tensor_shape), mybir.dt.np(alloc.dtype)))
    n_params = len(in_names)
    all_in_names = list(in_names) + out_names
    if partition_name is not None:
        all_in_names.append(partition_name)

    def _body(*args):
        operands = list(args)
        if partition_name is not None:
            operands.append(bass2jax.partition_id_tensor())
        outs = bass2jax._bass_exec_p.bind(
            *operands,
            out_avals=tuple(out_avals),
            in_names=tuple(all_in_names),
            out_names=tuple(out_names),
            lowering_input_output_aliases=(),
            sim_require_finite=True,
            sim_require_nnan=True,
            nc=nc,
        )
        return tuple(outs)

    devices = jax.devices()[:n_cores]
    mesh = Mesh(np.asarray(devices), ("core",))
    n_outs = len(out_avals)
    jitted = jax.jit(
        shard_map(_body, mesh=mesh,
                  in_specs=(PartitionSpec("core"),) * (n_params + n_outs),
                  out_specs=(PartitionSpec("core"),) * n_outs,
                  check_rep=False),
        keep_unused=True,
    )
    return jitted, in_names, out_names, out_avals


def make_core_inputs(anchor, positive, negative, similarity_matrix):
    """Host-side shard + dtype-cast. Returns {name: [per-core arrays]}."""
    import ml_dtypes
    f8 = np.dtype(mybir.dt.np(FP8))
    bf = np.dtype(mybir.dt.np(BF16))
    B, D = anchor.shape
    R = B // NCORES
    aT = np.ascontiguousarray(anchor.T.astype(f8))
    per = {"a_t": [aT] * NCORES, "anchor_local": [], "pos": [], "neg": [],
           "s": []}
    for c in range(NCORES):
        rows = slice(c * R, (c + 1) * R)
        per["anchor_local"].append(
            np.ascontiguousarray(anchor[rows]).astype(bf))
        per["pos"].append(np.ascontiguousarray(positive[rows]).astype(bf))
        per["neg"].append(np.ascontiguousarray(negative[rows]).astype(bf))
        per["s"].append(
            np.ascontiguousarray(similarity_matrix[rows]).astype(f8))
    return per


def run_cores(anchor, positive, negative, similarity_matrix, repeat=1):
    """Run the SPMD kernel, return per-core results list."""
    B, D = anchor.shape
    ncores = NCORES
    nc = _get_nc(B, D, ncores, repeat=repeat)
    per = make_core_inputs(anchor, positive, negative, similarity_matrix)

    key = (B, D, ncores, repeat)
    if key not in _jit_cache:
        _jit_cache[key] = _make_jit(nc, ncores)
    jitted, in_names, out_names, out_avals = _jit_cache[key]

    concat_in = [np.concatenate(per[n], axis=0) for n in in_names]
    concat_zeros = [np.zeros((ncores * a.shape[0], *a.shape[1:]), a.dtype)
                    for a in out_avals]
    out_arrs = jitted(*concat_in, *concat_zeros)
    return [
        {name: np.asarray(out_arrs[i]).reshape(ncores, *out_avals[i].shape)[c]
         for i, name in enumerate(out_names)}
        for c in range(ncores)
    ]


def combine(results, B):
    """Host-side reduction of the per-core partials (tiny)."""
    JW = min(2048, B)
    NG = B // JW
    QD = B // 512
    NS2 = NG * 2
    G = np.zeros((results[0]["g_out"].shape[0],) * 2, dtype=np.float64)
    qdot = 0.0
    s2 = 0.0
    trip = 0.0
    for r in results:
        G += r["g_out"].astype(np.float64)
        m = r["misc_out"].astype(np.float64)
        qdot += m[0, :QD].sum()          # replicated across partitions
        s2 += m[:, QD:QD + NS2].sum()
        trip += m[:, QD + NS2].sum()
    sum_cos2 = (G * G).sum()
    sim = (sum_cos2 - 2.0 * qdot + s2) / (float(B) ** 2)
    return np.asarray(trip / B + sim, dtype=np.float32)


def kernel(anchor, positive, negative, similarity_matrix):
    results = run_cores(anchor, positive, negative, similarity_matrix)
    return combine(results, anchor.shape[0])


# revision 27
# speedup vs baseline: 1.4681x; 1.4681x over previous
"""Trainium2 Bass kernel for nn_ContractiveLoss (triplet + pairwise-cosine MSE loss).

Math:
  triplet = mean(relu(||a-p+eps|| - ||a-n+eps|| + margin))
  sim     = mean((A_hat A_hat^T - S)^2),  A_hat = anchor rows normalized

The B x B cosine matrix is never materialized. Using
  sum((cos - S)^2) = sum(cos^2) - 2*sum(cos*S) + sum(S^2)
with
  sum(cos^2)  = ||G||_F^2,  G = A_hat^T A_hat           (D x D Gram)
  sum(cos*S)  = <S^T A_hat_local, A_hat>  per row-shard  (PE matmuls)
  sum(S^2)    = ACT Square with accumulate over S tiles
S (256 MiB) is the dominant HBM traffic and is read exactly once per core's
row-shard, cast fp32->bf16 during the DMA. The full anchor is also loaded
bf16-cast (only used for cosine-path values that are bf16-rounded anyway);
the local anchor/positive/negative stay fp32 for the triplet term.
Work is sharded row-wise across 8 NeuronCores; each core emits small
partials which are combined on host.

Emission order is tuned for overlap: local prep first (so PE matmuls can
start immediately), then anchor-normalize group g interleaved with
similarity column-group g (the group-g dot products need exactly the
group-g slice of the normalized anchor).

build(..., repeat=K) emits the body K times into one NEFF — used only for
timing (per-iteration steady-state period) since the axon path has no NTFF.
"""

import numpy as np

import concourse.bacc as bacc
import concourse.mybir as mybir
from concourse.tile import TileContext

F32 = mybir.dt.float32
BF16 = mybir.dt.bfloat16
AL = mybir.AluOpType
AF = mybir.ActivationFunctionType

MARGIN = 0.2
PD_EPS = 1e-6
COS_EPS = 1e-8

B_FULL, D_FULL, NCORES = 8192, 256, 8

_cache = {}


def _newton_sqrt(nc, scr_pool, y, x, cols):
    """y[:, cols] = sqrt(x[:, cols]), ACT sqrt + one Newton step.

    ACT Sqrt has a loose ULP budget; one step of y = 0.5*(y0 + x/y0)
    (with an accurate DVE reciprocal) squares the relative error.
    """
    p, n = y.shape[0], cols.stop - cols.start
    r = scr_pool.tile([p, n], F32, tag="nsq_r")
    nc.scalar.activation(out=y[:, cols], in_=x[:, cols], func=AF.Sqrt)
    nc.vector.reciprocal(out=r, in_=y[:, cols])
    # r = x / y0
    nc.vector.tensor_mul(out=r, in0=r, in1=x[:, cols])
    # y = (y0 * 1.0 + x/y0) * 0.5
    nc.vector.scalar_tensor_tensor(
        out=y[:, cols], in0=y[:, cols], scalar=1.0, in1=r,
        op0=AL.mult, op1=AL.add,
    )
    nc.vector.tensor_scalar_mul(out=y[:, cols], in0=y[:, cols], scalar1=0.5)


def _ssq_stt(nc, scr_pool, src, acc):
    """acc[:,0:1] = sum(src*src) along free dim (DVE fused square+reduce)."""
    sc = scr_pool.tile([128, src.shape[-1]], F32, tag="ssq_scr")
    nc.vector.scalar_tensor_tensor(
        out=sc, in0=src, scalar=0.0, in1=src,
        op0=AL.bypass, op1=AL.mult, accum_out=acc)


def build(B, D, ncores, repeat=1):
    """Build the per-core SPMD Bass module (identical NEFF on all cores)."""
    R = B // ncores          # local rows per core
    NT = B // 128            # 128-row tiles over all of B
    LT = R // 128            # local 128-row tiles
    JW = min(1024, B)        # similarity column-group width
    JG = B // JW             # number of column groups
    JC = JW // 128           # 128-col j-chunks per group
    AG = JG                  # anchor groups, one per column group
    GA = NT // AG            # anchor tiles per group
    assert NT % AG == 0 and D % 128 == 0 and R % 128 == 0 and B % JW == 0
    assert NT // AG == JC  # qdot indexes abf group tiles by j-chunk
    MC = NT + JG + 1         # misc cols: qdot per j-tile | s2 per group | triplet

    nc = bacc.Bacc("TRN2")
    anchor = nc.dram_tensor("anchor", [B, D], F32, kind="ExternalInput")
    anchor_l = nc.dram_tensor("anchor_local", [R, D], F32, kind="ExternalInput")
    pos = nc.dram_tensor("pos", [R, D], F32, kind="ExternalInput")
    neg = nc.dram_tensor("neg", [R, D], F32, kind="ExternalInput")
    s = nc.dram_tensor("s", [R, B], F32, kind="ExternalInput")
    g_out = nc.dram_tensor("g_out", [D, D], F32, kind="ExternalOutput")
    misc_out = nc.dram_tensor("misc_out", [128, MC], F32, kind="ExternalOutput")

    with TileContext(nc) as tc:
        with (
            tc.tile_pool(name="singles", bufs=1) as singles,
            tc.tile_pool(name="abf", bufs=4) as abf_pool,
            tc.tile_pool(name="stiles", bufs=6) as s_pool,
            tc.tile_pool(name="scr", bufs=4) as scr_pool,
            tc.tile_pool(name="qpsum", bufs=4, space="PSUM") as q_psum,
            tc.tile_pool(name="gpsum", bufs=2, space="PSUM") as g_psum,
        ):
            # persistent tiles (shared across repeats)
            misc = singles.tile([128, MC], F32)
            ssq = singles.tile([128, NT], F32)
            nrm = singles.tile([128, NT], F32)
            inv = singles.tile([128, NT], F32)
            al = singles.tile([128, LT, D], F32)
            albf = singles.tile([128, LT, D], BF16)
            pt = singles.tile([128, LT, D], F32)
            nt_ = singles.tile([128, LT, D], F32)
            ssql = singles.tile([128, LT], F32)
            nrml = singles.tile([128, LT], F32)
            invl = singles.tile([128, LT], F32)
            dp2 = singles.tile([128, LT], F32)
            dn2 = singles.tile([128, LT], F32)
            dpt = singles.tile([128, LT], F32)
            dnt = singles.tile([128, LT], F32)
            tm = singles.tile([128, LT], F32)
            rlu = singles.tile([128, LT], F32)
            g_sb = singles.tile([128, D // 128, D], F32)
            epsb = singles.tile([128, 1], F32)
            nc.vector.memset(epsb, PD_EPS)

            for _rep in range(repeat):
                # ------- local anchor prep + triplet (emitted first) -------
                nc.sync.dma_start(
                    out=al,
                    in_=anchor_l[:, :].rearrange("(t p) d -> p t d", p=128))
                nc.sync.dma_start(
                    out=pt, in_=pos[:, :].rearrange("(t p) d -> p t d", p=128))
                nc.sync.dma_start(
                    out=nt_, in_=neg[:, :].rearrange("(t p) d -> p t d", p=128))

                colsl = slice(0, LT)
                for i in range(LT):
                    _ssq_stt(nc, scr_pool, al[:, i, :], ssql[:, i:i + 1])
                _newton_sqrt(nc, scr_pool, nrml, ssql, colsl)
                nc.vector.tensor_scalar_max(out=nrml, in0=nrml, scalar1=COS_EPS)
                nc.vector.reciprocal(out=invl, in_=nrml)
                for i in range(LT):
                    nc.vector.tensor_scalar_mul(
                        out=albf[:, i, :], in0=al[:, i, :],
                        scalar1=invl[:, i:i + 1])

                # triplet term (all small; runs early while DMAs stream)
                for i in range(LT):
                    for (other, acc) in ((pt, dp2), (nt_, dn2)):
                        sc = scr_pool.tile([128, D], F32)
                        nc.vector.tensor_sub(
                            out=sc, in0=al[:, i, :], in1=other[:, i, :])
                        sc2 = scr_pool.tile([128, D], F32)
                        nc.scalar.activation(
                            out=sc2, in_=sc, func=AF.Square, bias=epsb[:, :],
                            scale=1.0, accum_out=acc[:, i:i + 1])
                _newton_sqrt(nc, scr_pool, dpt, dp2, colsl)
                _newton_sqrt(nc, scr_pool, dnt, dn2, colsl)
                # tm = (dpt + margin) - dnt ; then sum(relu(tm))
                nc.vector.scalar_tensor_tensor(
                    out=tm, in0=dpt, scalar=MARGIN, in1=dnt,
                    op0=AL.add, op1=AL.subtract)
                nc.vector.tensor_scalar(
                    out=rlu, in0=tm, scalar1=0.0, scalar2=None, op0=AL.max,
                    op1=AL.add, accum_out=misc[:, NT + JG:NT + JG + 1])

                # ------- local Gram G_c = Albf^T Albf (PE head-start) ------
                for h in range(D // 128):
                    gps = g_psum.tile([128, D], F32)
                    for i in range(LT):
                        nc.tensor.matmul(
                            out=gps,
                            lhsT=albf[:, i, h * 128:(h + 1) * 128],
                            rhs=albf[:, i, :],
                            start=(i == 0), stop=(i == LT - 1))
                    nc.vector.tensor_copy(out=g_sb[:, h, :], in_=gps)
                nc.sync.dma_start(
                    out=g_out[:, :].rearrange("(h p) k -> p h k", p=128),
                    in_=g_sb)

                # ------- main loop: anchor group g, then S group g ---------
                viewA = anchor[:, :].rearrange("(t p) d -> p t d", p=128)
                viewS = s[:, :].rearrange("(i p) j -> p i j", p=128)
                for g in range(JG):
                    # anchor-normalize group g (tiles t = g*GA .. g*GA+GA)
                    abf = abf_pool.tile([128, GA, D], BF16)
                    nc.gpsimd.dma_start(
                        out=abf, in_=viewA[:, g * GA:(g + 1) * GA, :])  # cast
                    for k in range(GA):
                        t = g * GA + k
                        _ssq_stt(nc, scr_pool, abf[:, k, :], ssq[:, t:t + 1])
                    cols = slice(g * GA, (g + 1) * GA)
                    _newton_sqrt(nc, scr_pool, nrm, ssq, cols)
                    nc.vector.tensor_scalar_max(
                        out=nrm[:, cols], in0=nrm[:, cols], scalar1=COS_EPS)
                    nc.vector.reciprocal(out=inv[:, cols], in_=nrm[:, cols])

                    # S column-group g: matmuls, dot with A_hat, sum of squares.
                    # The dot's A_hat[J] factor is raw bf16 anchor times the
                    # per-row 1/norm, folded in via the STT per-partition
                    # scalar: sum_k Q[j,k]*inv[j]*abf[j,k].
                    st = s_pool.tile([128, LT, JW], BF16)
                    nc.gpsimd.dma_start(
                        out=st, in_=viewS[:, :, g * JW:(g + 1) * JW])  # cast
                    for jj in range(JC):
                        J = g * JC + jj
                        qps = q_psum.tile([128, D], F32)
                        for i in range(LT):
                            nc.tensor.matmul(
                                out=qps,
                                lhsT=st[:, i, jj * 128:(jj + 1) * 128],
                                rhs=albf[:, i, :],
                                start=(i == 0), stop=(i == LT - 1))
                        qsc = scr_pool.tile([128, D], F32, tag="qdot_scr")
                        nc.vector.scalar_tensor_tensor(
                            out=qsc, in0=qps, scalar=inv[:, J:J + 1],
                            in1=abf[:, jj, :], op0=AL.mult, op1=AL.mult,
                            accum_out=misc[:, J:J + 1])
                    # in-place square of the (already consumed) S tile
                    nc.scalar.activation(
                        out=st, in_=st, func=AF.Square,
                        accum_out=misc[:, NT + g:NT + g + 1])

                nc.sync.dma_start(out=misc_out[:, :], in_=misc)

    nc.finalize()
    return nc


def _get_nc(B, D, ncores, repeat=1):
    key = (B, D, ncores, repeat)
    if key not in _cache:
        _cache[key] = build(B, D, ncores, repeat=repeat)
    return _cache[key]


_jit_cache = {}


def _make_jit(nc, n_cores):
    """Build a cached sharded jit around the bass_exec custom call (mirrors
    bass2jax.run_bass_via_pjrt, but reusable across kernel() invocations)."""
    import jax
    from jax.sharding import Mesh, PartitionSpec
    try:
        from jax.experimental.shard_map import shard_map
    except ImportError:
        from jax import shard_map
    import concourse.bass2jax as bass2jax

    bass2jax.install_neuronx_cc_hook()
    partition_name = (nc.partition_id_tensor.name
                      if nc.partition_id_tensor else None)
    in_names, out_names, out_avals = [], [], []
    for alloc in nc.m.functions[0].allocations:
        if not isinstance(alloc, mybir.MemoryLocationSet):
            continue
        name = alloc.memorylocations[0].name
        if alloc.kind == "ExternalInput":
            if name != partition_name:
                in_names.append(name)
        elif alloc.kind == "ExternalOutput":
            out_names.append(name)
            out_avals.append(jax.core.ShapedArray(
                tuple(alloc.tensor_shape), mybir.dt.np(alloc.dtype)))
    n_params = len(in_names)
    all_in_names = list(in_names) + out_names
    if partition_name is not None:
        all_in_names.append(partition_name)

    def _body(*args):
        operands = list(args)
        if partition_name is not None:
            operands.append(bass2jax.partition_id_tensor())
        outs = bass2jax._bass_exec_p.bind(
            *operands,
            out_avals=tuple(out_avals),
            in_names=tuple(all_in_names),
            out_names=tuple(out_names),
            lowering_input_output_aliases=(),
            sim_require_finite=True,
            sim_require_nnan=True,
            nc=nc,
        )
        return tuple(outs)

    devices = jax.devices()[:n_cores]
    mesh = Mesh(np.asarray(devices), ("core",))
    n_outs = len(out_avals)
    jitted = jax.jit(
        shard_map(_body, mesh=mesh,
                  in_specs=(PartitionSpec("core"),) * (n_params + n_outs),
                  out_specs=(PartitionSpec("core"),) * n_outs,
                  check_rep=False),
        keep_unused=True,
    )
    return jitted, in_names, out_names, out_avals


def run_cores(anchor, positive, negative, similarity_matrix, repeat=1):
    """Run the SPMD kernel, return per-core results list."""
    import jax
    B, D = anchor.shape
    ncores = NCORES
    R = B // ncores
    nc = _get_nc(B, D, ncores, repeat=repeat)
    anchor = np.ascontiguousarray(anchor, dtype=np.float32)
    in_maps = []
    for c in range(ncores):
        rows = slice(c * R, (c + 1) * R)
        in_maps.append({
            "anchor": anchor,
            "anchor_local": np.ascontiguousarray(anchor[rows]),
            "pos": np.ascontiguousarray(positive[rows], dtype=np.float32),
            "neg": np.ascontiguousarray(negative[rows], dtype=np.float32),
            "s": np.ascontiguousarray(similarity_matrix[rows], dtype=np.float32),
        })

    key = (B, D, ncores, repeat)
    if key not in _jit_cache:
        _jit_cache[key] = _make_jit(nc, ncores)
    jitted, in_names, out_names, out_avals = _jit_cache[key]

    concat_in = [np.concatenate([in_maps[c][n] for c in range(ncores)], axis=0)
                 for n in in_names]
    concat_zeros = [np.zeros((ncores * a.shape[0], *a.shape[1:]), a.dtype)
                    for a in out_avals]
    out_arrs = jitted(*concat_in, *concat_zeros)
    return [
        {name: np.asarray(out_arrs[i]).reshape(ncores, *out_avals[i].shape)[c]
         for i, name in enumerate(out_names)}
        for c in range(ncores)
    ]


def combine(results, B):
    """Host-side reduction of the per-core partials (tiny)."""
    NT = B // 128
    JG = B // min(1024, B)
    G = np.zeros((results[0]["g_out"].shape[0],) * 2, dtype=np.float64)
    qdot = 0.0
    s2 = 0.0
    trip = 0.0
    for r in results:
        G += r["g_out"].astype(np.float64)
        m = r["misc_out"].astype(np.float64)
        qdot += m[:, :NT].sum()
        s2 += m[:, NT:NT + JG].sum()
        trip += m[:, NT + JG].sum()
    sum_cos2 = (G * G).sum()
    sim = (sum_cos2 - 2.0 * qdot + s2) / (float(B) ** 2)
    return np.asarray(trip / B + sim, dtype=np.float32)


def kernel(anchor, positive, negative, similarity_matrix):
    results = run_cores(anchor, positive, negative, similarity_matrix)
    return combine(results, anchor.shape[0])

